# revision 3
# baseline (speedup 1.0000x reference)
"""MoE expert-pool kernel for Trainium2, 8 NeuronCores, expert+tensor parallel.

Strategy:
  - Host: route tokens to experts (distinct (token,expert) pairs, combined
    routing weight per pair). Pair up experts large+small; each core runs
    TWO half-FFN jobs: (expert_a, F-half h) and (expert_b, F-half 1-h), so
    per-core work is balanced at ~(C_a+C_b)/2 full-token equivalents.
  - Device (per job): yT_partial = W2h^T @ gelu(W1h^T @ XT + b1h) where
    W1h/W2h are the F-half slices. bf16 inputs, fp32 PSUM, bf16 partial out.
  - Host: sum the two half-F partials per expert, add b2, scale by combined
    weight, scatter-add into the [T, H] output.

Hardcoded problem shape: T=4096, H=1024, F=4096, E=8, K=2 (fp32 inputs).
"""

import sys
import types

import numpy as np
import ml_dtypes

H = 1024
F = 4096
FH = F // 2
E = 8
N_CORES = 8
PART = 128
TOK_CHUNK = 512  # fp32 PSUM bank = 512 columns

KT1 = H // PART    # 8  k-tiles for mm1 (contract over H)
MT1 = FH // PART   # 16 m-tiles for mm1 (output partitions = F-half chunks)
KT2 = FH // PART   # 16 k-tiles for mm2 (contract over F-half)
MT2 = H // PART    # 8  m-tiles for mm2 (output partitions = H chunks)

WARM_MMS = 56  # dummy matmuls to lift the HAM clock gate during DMA ramp


def _install_axon_trace_shim():
    """Make run_bass_kernel_spmd(trace=True) survive images that lack
    antenv.axon_hooks (tracing degrades gracefully if the hook .so is
    unavailable)."""
    try:
        import antenv.axon_hooks  # noqa: F401
        return
    except ImportError:
        pass
    mod = types.ModuleType("antenv.axon_hooks")
    mod._hook = None

    def set_axon_ntff_profile_hook(h):
        mod._hook = h

    def get_axon_ntff_profile_hook():
        return mod._hook

    mod.set_axon_ntff_profile_hook = set_axon_ntff_profile_hook
    mod.get_axon_ntff_profile_hook = get_axon_ntff_profile_hook
    sys.modules["antenv.axon_hooks"] = mod
    try:
        import antenv
        antenv.axon_hooks = mod
    except ImportError:
        pass
    try:
        from trn_agent_boot.trn_boot import _ntff_profile_via_ctypes
        mod._hook = _ntff_profile_via_ctypes("/opt/axon/libaxon_pjrt.so")
    except Exception:
        pass


_install_axon_trace_shim()

_PROGRAM_CACHE = {}


def _chunks_of(C):
    chunks = []
    off = 0
    while off < C:
        n = min(TOK_CHUNK, C - off)
        chunks.append((off, n))
        off += n
    return chunks


def _w1_groups():
    """W1-half DMA column groups: a small first group (one m-tile) so the
    first matmul group is gated by minimal bytes, then 512-wide groups."""
    groups = [(0, PART), (PART, 512 - PART)]
    groups += [(g, 512) for g in range(512, FH, 512)]
    return groups


def _w2_groups():
    return [(g, 512) for g in range(0, H, 512)]


def _pack_groups(w, kt, groups):
    """Pack a [kt*PART, cols] matrix into SBUF group-major layout
    [PART, kt*cols]: per group [p][(k, c)] contiguous."""
    w3 = w.reshape(kt, PART, w.shape[1])
    parts = [
        np.ascontiguousarray(
            w3[:, :, g0:g0 + gw].transpose(1, 0, 2).reshape(PART, kt * gw))
        for (g0, gw) in groups
    ]
    return np.ascontiguousarray(np.concatenate(parts, axis=1))


def _build_program(CA, CB):
    """Build + bacc-compile the per-core Bass program: two half-F expert
    jobs with token capacities CA (job a) and CB (job b)."""
    import concourse.mybir as mybir
    import concourse.tile as tile
    from concourse import bacc

    bf16 = mybir.dt.bfloat16
    f32 = mybir.dt.float32

    nc = bacc.Bacc("TRN2", target_bir_lowering=False, debug=False,
                   num_devices=N_CORES)

    w1_groups = _w1_groups()
    w2_groups = _w2_groups()

    # All inputs are host-arranged group-major in SBUF layout ([p][k][cols]
    # per group, groups concatenated) so every DMA reads fully-contiguous
    # per-partition lines.
    xa_d = nc.dram_tensor("xa", [PART, KT1 * CA], bf16, kind="ExternalInput")
    xb_d = nc.dram_tensor("xb", [PART, KT1 * CB], bf16, kind="ExternalInput")
    w1a_d = nc.dram_tensor("w1a", [PART, KT1 * FH], bf16, kind="ExternalInput")
    w1b_d = nc.dram_tensor("w1b", [PART, KT1 * FH], bf16, kind="ExternalInput")
    w2a_d = nc.dram_tensor("w2a", [PART, KT2 * H], bf16, kind="ExternalInput")
    w2b_d = nc.dram_tensor("w2b", [PART, KT2 * H], bf16, kind="ExternalInput")
    b1a_d = nc.dram_tensor("b1a", [PART, MT1], f32, kind="ExternalInput")
    b1b_d = nc.dram_tensor("b1b", [PART, MT1], f32, kind="ExternalInput")
    ya_d = nc.dram_tensor("ya", [H, CA], bf16, kind="ExternalOutput")
    yb_d = nc.dram_tensor("yb", [H, CB], bf16, kind="ExternalOutput")

    jobs = [
        (CA, _chunks_of(CA), xa_d, w1a_d, w2a_d, b1a_d, ya_d),
        (CB, _chunks_of(CB), xb_d, w1b_d, w2b_d, b1b_d, yb_d),
    ]

    with tile.TileContext(nc) as tc:
        with (
            tc.tile_pool(name="big", bufs=1) as big_pool,
            tc.tile_pool(name="consts", bufs=1) as consts,
            tc.tile_pool(name="stage", bufs=4) as stage_pool,
            tc.tile_pool(name="psum", bufs=4, space="PSUM") as psum_pool,
            tc.tile_pool(name="wpsum", bufs=1, space="PSUM") as wpsum_pool,
        ):
            gelu = mybir.ActivationFunctionType.Gelu

            # PE pre-warm: zero-tile matmuls keep the PE busy through the
            # HAM activity window so the real stream starts at 2.4 GHz.
            warm_sb = consts.tile([PART, PART], bf16)
            nc.vector.memset(warm_sb[:], 0.0)
            wps = wpsum_pool.tile([PART, PART], f32)
            for _ in range(WARM_MMS):
                nc.tensor.matmul(wps[:], warm_sb[:], warm_sb[:],
                                 start=True, stop=True)

            # SBUF tiles mirror the DRAM packed layout exactly.
            sb = []
            for j, (C, chunks, x_d, w1_d, w2_d, b1_d, y_d) in enumerate(jobs):
                sb.append({
                    "x": big_pool.tile([PART, KT1 * C], bf16,
                                       name=f"x{j}", tag=f"x{j}"),
                    "w1": big_pool.tile([PART, KT1 * FH], bf16,
                                        name=f"w1{j}", tag=f"w1{j}"),
                    "w2": big_pool.tile([PART, KT2 * H], bf16,
                                        name=f"w2{j}", tag=f"w2{j}"),
                    "b1": consts.tile([PART, MT1], f32,
                                      name=f"b1{j}", tag=f"b1{j}"),
                })
            h_sb = big_pool.tile([PART, MT1, TOK_CHUNK], bf16)

            # DMA order = consumption order. Critical prefix (gates the
            # first matmul group): job-a chunk-0 tokens split per k-slab
            # (parallel queues) + W1a's first m-tile.
            (t00, tn0) = jobs[0][1][0]
            for k in range(KT1):
                nc.sync.dma_start(
                    sb[0]["x"][:, t00 * KT1 + k * tn0:
                               t00 * KT1 + (k + 1) * tn0],
                    xa_d.ap()[:, t00 * KT1 + k * tn0:
                              t00 * KT1 + (k + 1) * tn0])
            for gi, (g0, gw) in enumerate(w1_groups):
                eng = nc.gpsimd if gi == 0 else nc.sync
                eng.dma_start(sb[0]["w1"][:, g0 * KT1:(g0 + gw) * KT1],
                              w1a_d.ap()[:, g0 * KT1:(g0 + gw) * KT1])
                if gi == 1:
                    nc.gpsimd.dma_start(sb[0]["b1"][:], b1a_d.ap())
            for (t0, tn) in jobs[0][1][1:]:
                nc.sync.dma_start(sb[0]["x"][:, t0 * KT1:(t0 + tn) * KT1],
                                  xa_d.ap()[:, t0 * KT1:(t0 + tn) * KT1])
            for (g0, gw) in w2_groups:
                nc.sync.dma_start(sb[0]["w2"][:, g0 * KT2:(g0 + gw) * KT2],
                                  w2a_d.ap()[:, g0 * KT2:(g0 + gw) * KT2])
            # job b inputs (consumed in the second half of the kernel)
            for (t0, tn) in jobs[1][1]:
                nc.sync.dma_start(sb[1]["x"][:, t0 * KT1:(t0 + tn) * KT1],
                                  xb_d.ap()[:, t0 * KT1:(t0 + tn) * KT1])
            for gi, (g0, gw) in enumerate(w1_groups):
                nc.sync.dma_start(sb[1]["w1"][:, g0 * KT1:(g0 + gw) * KT1],
                                  w1b_d.ap()[:, g0 * KT1:(g0 + gw) * KT1])
                if gi == 0:
                    nc.gpsimd.dma_start(sb[1]["b1"][:], b1b_d.ap())
            for (g0, gw) in w2_groups:
                nc.sync.dma_start(sb[1]["w2"][:, g0 * KT2:(g0 + gw) * KT2],
                                  w2b_d.ap()[:, g0 * KT2:(g0 + gw) * KT2])

            def x_slice(x_sb, t0, tn, k):
                # tokens [t0, t0+tn) of k-slab k (chunk-major packing)
                base = t0 * KT1 + k * tn
                return x_sb[:, base:base + tn]

            def w_slice(w_sb, groups, kt, m, k):
                # m-tile m, k-slab k from group-major packing
                for (g0, gw) in groups:
                    if g0 <= m * PART < g0 + gw:
                        base = g0 * kt + k * gw + (m * PART - g0)
                        return w_sb[:, base:base + PART]
                raise AssertionError

            for j, (C, chunks, x_d, w1_d, w2_d, b1_d, y_d) in enumerate(jobs):
                s = sb[j]
                for (t0, tn) in chunks:
                    # mm1 + gelu: h = gelu(W1h^T X + b1h) for this chunk
                    for m in range(MT1):
                        ps = psum_pool.tile([PART, TOK_CHUNK], f32, tag="ps",
                                            name="ps")
                        for k in range(KT1):
                            nc.tensor.matmul(
                                ps[:, :tn],
                                w_slice(s["w1"], w1_groups, KT1, m, k),
                                x_slice(s["x"], t0, tn, k),
                                start=(k == 0), stop=(k == KT1 - 1))
                        nc.scalar.activation(
                            h_sb[:, m, :tn], ps[:, :tn], gelu,
                            bias=s["b1"][:, m:m + 1], scale=1.0)

                    # mm2: yt_partial = W2h^T h for this token chunk
                    for m in range(MT2):
                        ps = psum_pool.tile([PART, TOK_CHUNK], f32, tag="ps",
                                            name="ps")
                        for k in range(KT2):
                            nc.tensor.matmul(
                                ps[:, :tn],
                                w_slice(s["w2"], w2_groups, KT2, m, k),
                                h_sb[:, k, :tn],
                                start=(k == 0), stop=(k == KT2 - 1))
                        out_sb = stage_pool.tile([PART, TOK_CHUNK], bf16,
                                                 tag="out", name="out")
                        last = (j == 1 and m == MT2 - 1 and t0 + tn >= C)
                        if last:
                            # tail-critical: copy+DMA in quarters so DMAs
                            # overlap the remaining copies
                            qs = [(tn * i // 4, tn * (i + 1) // 4)
                                  for i in range(4)]
                            for (a, b) in qs:
                                nc.vector.tensor_copy(out_sb[:, a:b],
                                                      ps[:, a:b])
                                nc.sync.dma_start(
                                    y_d.ap()[m * PART:(m + 1) * PART,
                                             t0 + a:t0 + b],
                                    out_sb[:, a:b])
                        else:
                            nc.vector.tensor_copy(out_sb[:, :tn], ps[:, :tn])
                            nc.sync.dma_start(
                                y_d.ap()[m * PART:(m + 1) * PART, t0:t0 + tn],
                                out_sb[:, :tn])

    nc.compile()
    return nc


def _route(expert_weights, selected_experts):
    """Distinct (token, expert) pairs with combined weights.

    Returns per-expert (token_ids, combined_weights)."""
    se = np.asarray(selected_experts).astype(np.int64)
    ew = np.asarray(expert_weights).astype(np.float32)
    routes = []
    for e in range(E):
        hit = (se == e)  # [T, K]
        tok = np.nonzero(hit.any(axis=1))[0]
        cw = (ew * hit).sum(axis=1)[tok]
        routes.append((tok, cw))
    return routes


def _pack_x(hs, tok, C):
    """Pack tokens into the chunk-major [PART, KT1*C] bf16 device layout."""
    bf16 = ml_dtypes.bfloat16
    xt = np.zeros((H, C), dtype=bf16)
    if len(tok):
        xt[:, :len(tok)] = hs[tok].T.astype(bf16)
    return _pack_groups(xt, KT1, _chunks_of(C))


def kernel(hidden_states, expert_weights, W1, b1, W2, b2, selected_experts):
    from concourse.bass_utils import run_bass_kernel_spmd

    hs = np.asarray(hidden_states)
    out_dtype = hs.dtype
    hs = hs.astype(np.float32)
    W1 = np.asarray(W1).astype(np.float32)
    b1 = np.asarray(b1).astype(np.float32)
    W2 = np.asarray(W2).astype(np.float32)
    b2 = np.asarray(b2).astype(np.float32)

    T = hs.shape[0]
    assert hs.shape[1] == H and W1.shape == (E, H, F) and W2.shape == (E, F, H)

    routes = _route(expert_weights, selected_experts)
    counts = np.array([len(tok) for tok, _ in routes])

    # Pair experts large+small: pairs (order[i], order[7-i]); per pair two
    # cores, one per F-half orientation.
    order = np.argsort(-counts, kind="stable")
    pairs = [(int(order[i]), int(order[7 - i])) for i in range(4)]
    CA = max(PART, int(max(counts[a] for a, _ in pairs)))
    CB = max(PART, int(max(counts[b] for _, b in pairs)))

    if (CA, CB) not in _PROGRAM_CACHE:
        _PROGRAM_CACHE[(CA, CB)] = _build_program(CA, CB)
    nc = _PROGRAM_CACHE[(CA, CB)]

    bf16 = ml_dtypes.bfloat16
    w1_groups = _w1_groups()
    w2_groups = _w2_groups()

    # Per-expert packed halves (shared between the two cores of a pair).
    xa_cache = {}
    w1h = {}
    w2h = {}
    b1h = {}
    for e in set(e for p in pairs for e in p):
        for half in (0, 1):
            c0, c1 = half * FH, (half + 1) * FH
            w1h[(e, half)] = _pack_groups(
                W1[e][:, c0:c1].astype(bf16), KT1, w1_groups)
            w2h[(e, half)] = _pack_groups(
                W2[e][c0:c1, :].astype(bf16), KT2, w2_groups)
            b1h[(e, half)] = np.ascontiguousarray(
                b1[e][c0:c1].reshape(MT1, PART).T)

    in_maps = []
    core_jobs = []  # per core: (expert_a, half_a, expert_b, half_b)
    for (a, b) in pairs:
        if a not in xa_cache:
            xa_cache[a] = _pack_x(hs, routes[a][0], CA)
        if b not in xa_cache:
            xa_cache[b] = _pack_x(hs, routes[b][0], CB)
        for half in (0, 1):
            in_maps.append({
                "xa": xa_cache[a],
                "xb": xa_cache[b],
                "w1a": w1h[(a, half)],
                "w2a": w2h[(a, half)],
                "b1a": b1h[(a, half)],
                "w1b": w1h[(b, 1 - half)],
                "w2b": w2h[(b, 1 - half)],
                "b1b": b1h[(b, 1 - half)],
            })
            core_jobs.append((a, half, b, 1 - half))

    res = run_bass_kernel_spmd(nc, in_maps, core_ids=list(range(N_CORES)))

    out = np.zeros((T, H), dtype=np.float32)
    for pi, (a, b) in enumerate(pairs):
        r0 = res.results[2 * pi]
        r1 = res.results[2 * pi + 1]
        for e, key in ((a, "ya"), (b, "yb")):
            tok, cw = routes[e]
            if len(tok) == 0:
                continue
            yt = (r0[key][:, :len(tok)].astype(np.float32)
                  + r1[key][:, :len(tok)].astype(np.float32))
            out[tok] += cw[:, None] * (yt.T + b2[e][None, :])
    return out.astype(out_dtype)


# revision 7
# speedup vs baseline: 1.0302x; 1.0302x over previous
"""MoE expert-pool kernel for Trainium2, 8 NeuronCores, expert+tensor parallel.

Strategy:
  - Host: route tokens to experts (distinct (token,expert) pairs, combined
    routing weight per pair). Pair up experts large+small; each core runs
    TWO half-FFN jobs: (expert_a, F-half h) and (expert_b, F-half 1-h), so
    per-core work is balanced at ~(C_a+C_b)/2 full-token equivalents.
  - Device (per job): yT_partial = W2h^T @ gelu(W1h^T @ XT + b1h) where
    W1h/W2h are the F-half slices. bf16 inputs, fp32 PSUM, bf16 partial out.
  - Host: sum the two half-F partials per expert, add b2, scale by combined
    weight, scatter-add into the [T, H] output.

Hardcoded problem shape: T=4096, H=1024, F=4096, E=8, K=2 (fp32 inputs).
"""

import sys
import types

import numpy as np
import ml_dtypes

H = 1024
F = 4096
FH = F // 2
E = 8
N_CORES = 8
PART = 128
TOK_CHUNK = 512  # fp32 PSUM bank = 512 columns

KT1 = H // PART    # 8  k-tiles for mm1 (contract over H)
MT1 = FH // PART   # 16 m-tiles for mm1 (output partitions = F-half chunks)
KT2 = FH // PART   # 16 k-tiles for mm2 (contract over F-half)
MT2 = H // PART    # 8  m-tiles for mm2 (output partitions = H chunks)

WARM_MMS = 32  # dummy matmuls to lift the HAM clock gate during DMA ramp


def _install_axon_trace_shim():
    """Make run_bass_kernel_spmd(trace=True) survive images that lack
    antenv.axon_hooks (tracing degrades gracefully if the hook .so is
    unavailable)."""
    try:
        import antenv.axon_hooks  # noqa: F401
        return
    except ImportError:
        pass
    mod = types.ModuleType("antenv.axon_hooks")
    mod._hook = None

    def set_axon_ntff_profile_hook(h):
        mod._hook = h

    def get_axon_ntff_profile_hook():
        return mod._hook

    mod.set_axon_ntff_profile_hook = set_axon_ntff_profile_hook
    mod.get_axon_ntff_profile_hook = get_axon_ntff_profile_hook
    sys.modules["antenv.axon_hooks"] = mod
    try:
        import antenv
        antenv.axon_hooks = mod
    except ImportError:
        pass
    try:
        from trn_agent_boot.trn_boot import _ntff_profile_via_ctypes
        mod._hook = _ntff_profile_via_ctypes("/opt/axon/libaxon_pjrt.so")
    except Exception:
        pass


_install_axon_trace_shim()

_PROGRAM_CACHE = {}


def _chunks_of(C):
    chunks = []
    off = 0
    while off < C:
        n = min(TOK_CHUNK, C - off)
        chunks.append((off, n))
        off += n
    return chunks


def _w1_groups():
    """W1-half DMA column groups: a small first group (one m-tile) so the
    first matmul group is gated by minimal bytes, then 512-wide groups."""
    groups = [(0, PART), (PART, 512 - PART)]
    groups += [(g, 512) for g in range(512, FH, 512)]
    return groups


def _w2_groups():
    return [(g, 512) for g in range(0, H, 512)]


def _pack_groups(w, kt, groups):
    """Pack a [kt*PART, cols] matrix into SBUF group-major layout
    [PART, kt*cols]: per group [p][(k, c)] contiguous."""
    w3 = w.reshape(kt, PART, w.shape[1])
    parts = [
        np.ascontiguousarray(
            w3[:, :, g0:g0 + gw].transpose(1, 0, 2).reshape(PART, kt * gw))
        for (g0, gw) in groups
    ]
    return np.ascontiguousarray(np.concatenate(parts, axis=1))


def _build_program(CA, CB):
    """Build + bacc-compile the per-core Bass program: two half-F expert
    jobs with token capacities CA (job a) and CB (job b)."""
    import concourse.mybir as mybir
    import concourse.tile as tile
    from concourse import bacc

    bf16 = mybir.dt.bfloat16
    f32 = mybir.dt.float32

    nc = bacc.Bacc("TRN2", target_bir_lowering=False, debug=False,
                   num_devices=N_CORES)

    w1_groups = _w1_groups()
    w2_groups = _w2_groups()

    # All inputs are host-arranged group-major in SBUF layout ([p][k][cols]
    # per group, groups concatenated) so every DMA reads fully-contiguous
    # per-partition lines.
    xa_d = nc.dram_tensor("xa", [PART, KT1 * CA], bf16, kind="ExternalInput")
    xb_d = nc.dram_tensor("xb", [PART, KT1 * CB], bf16, kind="ExternalInput")
    w1a_d = nc.dram_tensor("w1a", [PART, KT1 * FH], bf16, kind="ExternalInput")
    w1b_d = nc.dram_tensor("w1b", [PART, KT1 * FH], bf16, kind="ExternalInput")
    w2a_d = nc.dram_tensor("w2a", [PART, KT2 * H], bf16, kind="ExternalInput")
    w2b_d = nc.dram_tensor("w2b", [PART, KT2 * H], bf16, kind="ExternalInput")
    b1a_d = nc.dram_tensor("b1a", [PART, MT1], f32, kind="ExternalInput")
    b1b_d = nc.dram_tensor("b1b", [PART, MT1], f32, kind="ExternalInput")
    ya_d = nc.dram_tensor("ya", [H, CA], bf16, kind="ExternalOutput")
    yb_d = nc.dram_tensor("yb", [H, CB], bf16, kind="ExternalOutput")

    jobs = [
        (CA, _chunks_of(CA), xa_d, w1a_d, w2a_d, b1a_d, ya_d),
        (CB, _chunks_of(CB), xb_d, w1b_d, w2b_d, b1b_d, yb_d),
    ]

    with tile.TileContext(nc) as tc:
        with (
            tc.tile_pool(name="big", bufs=1) as big_pool,
            tc.tile_pool(name="consts", bufs=1) as consts,
            tc.tile_pool(name="stage", bufs=8) as stage_pool,
            tc.tile_pool(name="psum", bufs=4, space="PSUM") as psum_pool,
            tc.tile_pool(name="wpsum", bufs=1, space="PSUM") as wpsum_pool,
        ):
            gelu = mybir.ActivationFunctionType.Gelu

            # PE pre-warm: zero-tile matmuls keep the PE busy through the
            # HAM activity window so the real stream starts at 2.4 GHz.
            warm_sb = consts.tile([PART, PART], bf16)
            nc.vector.memset(warm_sb[:], 0.0)
            wps = wpsum_pool.tile([PART, PART], f32)
            for _ in range(WARM_MMS):
                nc.tensor.matmul(wps[:], warm_sb[:], warm_sb[:],
                                 start=True, stop=True)

            # SBUF tiles mirror the DRAM packed layout exactly.
            sb = []
            for j, (C, chunks, x_d, w1_d, w2_d, b1_d, y_d) in enumerate(jobs):
                sb.append({
                    "x": big_pool.tile([PART, KT1 * C], bf16,
                                       name=f"x{j}", tag=f"x{j}"),
                    "w1": big_pool.tile([PART, KT1 * FH], bf16,
                                        name=f"w1{j}", tag=f"w1{j}"),
                    "w2": big_pool.tile([PART, KT2 * H], bf16,
                                        name=f"w2{j}", tag=f"w2{j}"),
                    "b1": consts.tile([PART, MT1], f32,
                                      name=f"b1{j}", tag=f"b1{j}"),
                })
            h_sb = big_pool.tile([PART, MT1, TOK_CHUNK], bf16)

            # Input DMAs all on sync (its own 16 HW rings), in consumption
            # order. Output DMAs + biases go on scalar's separate rings so
            # they are never queued behind the input burst. Critical prefix:
            # W1a's first m-tile, then job-a chunk-0 tokens in two pieces.
            (t00, tn0) = jobs[0][1][0]
            (g0_, gw_) = w1_groups[0]
            nc.sync.dma_start(sb[0]["w1"][:, g0_ * KT1:(g0_ + gw_) * KT1],
                              w1a_d.ap()[:, g0_ * KT1:(g0_ + gw_) * KT1])
            half_k = KT1 // 2
            for (ka, kb) in ((0, half_k), (half_k, KT1)):
                nc.sync.dma_start(
                    sb[0]["x"][:, t00 * KT1 + ka * tn0:
                               t00 * KT1 + kb * tn0],
                    xa_d.ap()[:, t00 * KT1 + ka * tn0:
                              t00 * KT1 + kb * tn0])
            nc.scalar.dma_start(sb[0]["b1"][:], b1a_d.ap())
            nc.scalar.dma_start(sb[1]["b1"][:], b1b_d.ap())
            for (g0, gw) in w1_groups[1:]:
                nc.sync.dma_start(sb[0]["w1"][:, g0 * KT1:(g0 + gw) * KT1],
                                  w1a_d.ap()[:, g0 * KT1:(g0 + gw) * KT1])
            for (t0, tn) in jobs[0][1][1:]:
                nc.sync.dma_start(sb[0]["x"][:, t0 * KT1:(t0 + tn) * KT1],
                                  xa_d.ap()[:, t0 * KT1:(t0 + tn) * KT1])
            for (g0, gw) in w2_groups:
                nc.sync.dma_start(sb[0]["w2"][:, g0 * KT2:(g0 + gw) * KT2],
                                  w2a_d.ap()[:, g0 * KT2:(g0 + gw) * KT2])
            # job b inputs (consumed in the second half of the kernel)
            for (t0, tn) in jobs[1][1]:
                nc.sync.dma_start(sb[1]["x"][:, t0 * KT1:(t0 + tn) * KT1],
                                  xb_d.ap()[:, t0 * KT1:(t0 + tn) * KT1])
            for (g0, gw) in w1_groups:
                nc.sync.dma_start(sb[1]["w1"][:, g0 * KT1:(g0 + gw) * KT1],
                                  w1b_d.ap()[:, g0 * KT1:(g0 + gw) * KT1])
            for (g0, gw) in w2_groups:
                nc.sync.dma_start(sb[1]["w2"][:, g0 * KT2:(g0 + gw) * KT2],
                                  w2b_d.ap()[:, g0 * KT2:(g0 + gw) * KT2])

            def x_slice(x_sb, t0, tn, k):
                # tokens [t0, t0+tn) of k-slab k (chunk-major packing)
                base = t0 * KT1 + k * tn
                return x_sb[:, base:base + tn]

            def w_slice(w_sb, groups, kt, m, k):
                # m-tile m, k-slab k from group-major packing
                for (g0, gw) in groups:
                    if g0 <= m * PART < g0 + gw:
                        base = g0 * kt + k * gw + (m * PART - g0)
                        return w_sb[:, base:base + PART]
                raise AssertionError

            for j, (C, chunks, x_d, w1_d, w2_d, b1_d, y_d) in enumerate(jobs):
                s = sb[j]
                for (t0, tn) in chunks:
                    # mm1 + gelu: h = gelu(W1h^T X + b1h) for this chunk
                    for m in range(MT1):
                        ps = psum_pool.tile([PART, TOK_CHUNK], f32, tag="ps",
                                            name="ps")
                        for k in range(KT1):
                            nc.tensor.matmul(
                                ps[:, :tn],
                                w_slice(s["w1"], w1_groups, KT1, m, k),
                                x_slice(s["x"], t0, tn, k),
                                start=(k == 0), stop=(k == KT1 - 1))
                        nc.scalar.activation(
                            h_sb[:, m, :tn], ps[:, :tn], gelu,
                            bias=s["b1"][:, m:m + 1], scale=1.0)

                    # mm2: yt_partial = W2h^T h for this token chunk
                    for m in range(MT2):
                        ps = psum_pool.tile([PART, TOK_CHUNK], f32, tag="ps",
                                            name="ps")
                        for k in range(KT2):
                            nc.tensor.matmul(
                                ps[:, :tn],
                                w_slice(s["w2"], w2_groups, KT2, m, k),
                                h_sb[:, k, :tn],
                                start=(k == 0), stop=(k == KT2 - 1))
                        out_sb = stage_pool.tile([PART, TOK_CHUNK], bf16,
                                                 tag="out", name="out")
                        last = (j == 1 and m == MT2 - 1 and t0 + tn >= C)
                        if last:
                            # tail-critical: copy+DMA in quarters so DMAs
                            # overlap the remaining copies
                            qs = [(tn * i // 4, tn * (i + 1) // 4)
                                  for i in range(4)]
                            for (a, b) in qs:
                                nc.vector.tensor_copy(out_sb[:, a:b],
                                                      ps[:, a:b])
                                nc.scalar.dma_start(
                                    y_d.ap()[m * PART:(m + 1) * PART,
                                             t0 + a:t0 + b],
                                    out_sb[:, a:b])
                        else:
                            nc.vector.tensor_copy(out_sb[:, :tn], ps[:, :tn])
                            nc.scalar.dma_start(
                                y_d.ap()[m * PART:(m + 1) * PART, t0:t0 + tn],
                                out_sb[:, :tn])

    nc.compile()
    return nc


def _route(expert_weights, selected_experts):
    """Distinct (token, expert) pairs with combined weights.

    Returns per-expert (token_ids, combined_weights)."""
    se = np.asarray(selected_experts).astype(np.int64)
    ew = np.asarray(expert_weights).astype(np.float32)
    routes = []
    for e in range(E):
        hit = (se == e)  # [T, K]
        tok = np.nonzero(hit.any(axis=1))[0]
        cw = (ew * hit).sum(axis=1)[tok]
        routes.append((tok, cw))
    return routes


def _pack_x(hs, tok, C):
    """Pack tokens into the chunk-major [PART, KT1*C] bf16 device layout."""
    bf16 = ml_dtypes.bfloat16
    xt = np.zeros((H, C), dtype=bf16)
    if len(tok):
        xt[:, :len(tok)] = hs[tok].T.astype(bf16)
    return _pack_groups(xt, KT1, _chunks_of(C))


def kernel(hidden_states, expert_weights, W1, b1, W2, b2, selected_experts):
    from concourse.bass_utils import run_bass_kernel_spmd

    hs = np.asarray(hidden_states)
    out_dtype = hs.dtype
    hs = hs.astype(np.float32)
    W1 = np.asarray(W1).astype(np.float32)
    b1 = np.asarray(b1).astype(np.float32)
    W2 = np.asarray(W2).astype(np.float32)
    b2 = np.asarray(b2).astype(np.float32)

    T = hs.shape[0]
    assert hs.shape[1] == H and W1.shape == (E, H, F) and W2.shape == (E, F, H)

    routes = _route(expert_weights, selected_experts)
    counts = np.array([len(tok) for tok, _ in routes])

    # Pair experts large+small: pairs (order[i], order[7-i]); per pair two
    # cores, one per F-half orientation.
    order = np.argsort(-counts, kind="stable")
    pairs = [(int(order[i]), int(order[7 - i])) for i in range(4)]
    CA = max(PART, int(max(counts[a] for a, _ in pairs)))
    CB = max(PART, int(max(counts[b] for _, b in pairs)))

    if (CA, CB) not in _PROGRAM_CACHE:
        _PROGRAM_CACHE[(CA, CB)] = _build_program(CA, CB)
    nc = _PROGRAM_CACHE[(CA, CB)]

    bf16 = ml_dtypes.bfloat16
    w1_groups = _w1_groups()
    w2_groups = _w2_groups()

    # Per-expert packed halves (shared between the two cores of a pair).
    xa_cache = {}
    w1h = {}
    w2h = {}
    b1h = {}
    for e in set(e for p in pairs for e in p):
        for half in (0, 1):
            c0, c1 = half * FH, (half + 1) * FH
            w1h[(e, half)] = _pack_groups(
                W1[e][:, c0:c1].astype(bf16), KT1, w1_groups)
            w2h[(e, half)] = _pack_groups(
                W2[e][c0:c1, :].astype(bf16), KT2, w2_groups)
            b1h[(e, half)] = np.ascontiguousarray(
                b1[e][c0:c1].reshape(MT1, PART).T)

    in_maps = []
    core_jobs = []  # per core: (expert_a, half_a, expert_b, half_b)
    for (a, b) in pairs:
        if a not in xa_cache:
            xa_cache[a] = _pack_x(hs, routes[a][0], CA)
        if b not in xa_cache:
            xa_cache[b] = _pack_x(hs, routes[b][0], CB)
        for half in (0, 1):
            in_maps.append({
                "xa": xa_cache[a],
                "xb": xa_cache[b],
                "w1a": w1h[(a, half)],
                "w2a": w2h[(a, half)],
                "b1a": b1h[(a, half)],
                "w1b": w1h[(b, 1 - half)],
                "w2b": w2h[(b, 1 - half)],
                "b1b": b1h[(b, 1 - half)],
            })
            core_jobs.append((a, half, b, 1 - half))

    res = run_bass_kernel_spmd(nc, in_maps, core_ids=list(range(N_CORES)))

    out = np.zeros((T, H), dtype=np.float32)
    for pi, (a, b) in enumerate(pairs):
        r0 = res.results[2 * pi]
        r1 = res.results[2 * pi + 1]
        for e, key in ((a, "ya"), (b, "yb")):
            tok, cw = routes[e]
            if len(tok) == 0:
                continue
            yt = (r0[key][:, :len(tok)].astype(np.float32)
                  + r1[key][:, :len(tok)].astype(np.float32))
            out[tok] += cw[:, None] * (yt.T + b2[e][None, :])
    return out.astype(out_dtype)
